# revision 1
# baseline (speedup 1.0000x reference)
"""GQA attention kernel for Trainium2, 8 NeuronCores.

Problem: B=2, T=2048, E=2048, 32 query heads, 8 KV heads, head_dim=64, causal.
Sharding: 2 (batch) x 4 (tensor-parallel) grid. Each TP rank owns 2 KV heads
(=> 8 query heads, 512 q-channels). Wq/Wkv column-sharded, Wo row-sharded;
per-rank bf16 partial outputs are summed on host.

Design (355us vs 578us v1 baseline; PE-bound, HAM-warmth driven):
 - single-pass pipelined structure over four 512-wide query windows: Q/K/V
   projections for later windows and O-projections of completed windows are
   generator-woven into the attention matmul stream (pump/drain scheduler
   with hard deadlines for QKV and rationed O budgets) so the PE never sits
   idle long enough for the HAM clock gate to re-throttle it to 1.2 GHz.
 - score matmuls for the two KV halves run CONCURRENTLY in the PE array via
   row tiling (tile_position (0,0)/(64,0); contraction = head_dim = 64).
 - exp batched per k-block across both halves (one ACTIVATE on [128, 2x512]),
   diagonal blocks sliced to live columns; causal tri-mask on DVE.
 - attention outputs accumulate into PSUM [65,512] per half with a ones
   column producing softmax denominators for free; denominators from all 8
   heads of a window are gathered to partitions 32g via small DMAs so ONE
   DVE reciprocal per window normalizes everything (v1 spent 107us here).
 - reciprocal broadcast back across partitions with rank-1 matmuls (all-ones
   lhsT row 32g); the [128,512] PSUM broadcast feeds the DVE normalize
   multiply directly as its one allowed PSUM operand.
 - each window's normalization is emitted inside the NEXT window's attention
   stream (reciprocal at window end, broadcasts/multiplies after the next
   window's first head-pair) to keep window boundaries matmul-dense.
 - output written as bf16 partials (halves output DMA traffic).
"""

import numpy as np
import ml_dtypes

import concourse.bass as bass
import concourse.mybir as mybir
import concourse.tile as tile
from concourse import bacc
from concourse.bass_utils import run_bass_kernel_spmd

E = 2048
T = 2048
HQ = 32
HKV = 8
HD = 64
G = 4            # query heads per kv head
P = 128
QL = 512         # local q channels per rank (8 heads)
KVL = 128        # local k (or v) channels per rank (2 heads)
NB = 2           # batches
NR = 4           # tensor-parallel ranks
SCALE = 1.0 / 8.0
EC = E // P      # 16 contraction chunks
NW = 4           # number of 512-wide t windows

BF16 = mybir.dt.bfloat16
F32 = mybir.dt.float32

_CACHE = {}


def _build_program():
    from contextlib import ExitStack

    nc = bacc.Bacc(None, target_bir_lowering=False, debug=False)
    xT = nc.declare_dram_parameter("xT", [E, T], BF16, isOutput=False)
    wqT = nc.declare_dram_parameter("wqT", [E, QL], BF16, isOutput=False)
    wkvT = nc.declare_dram_parameter("wkvT", [E, 2 * KVL], BF16, isOutput=False)
    woT = nc.declare_dram_parameter("woT", [QL, E], BF16, isOutput=False)
    tri = nc.declare_dram_parameter("tri", [P, P], BF16, isOutput=False)
    out = nc.declare_dram_parameter("out", [T, E], BF16, isOutput=True)
    out_r = out.rearrange("(o p) e -> p o e", p=P)

    with tile.TileContext(nc) as tc, ExitStack() as ctx:
        const = ctx.enter_context(tc.tile_pool(name="const", bufs=1))
        qtp = ctx.enter_context(tc.tile_pool(name="qtp", bufs=4))
        atp = ctx.enter_context(tc.tile_pool(name="atp", bufs=4))
        atu = ctx.enter_context(tc.tile_pool(name="atu", bufs=9))
        stg = ctx.enter_context(tc.tile_pool(name="stg", bufs=3))
        ptp = ctx.enter_context(tc.tile_pool(name="ptp", bufs=3))
        dsb = ctx.enter_context(tc.tile_pool(name="dsb", bufs=3))
        d8p = ctx.enter_context(tc.tile_pool(name="d8p", bufs=2))
        rcp = ctx.enter_context(tc.tile_pool(name="rcp", bufs=2))
        bcp = ctx.enter_context(tc.tile_pool(name="bcp", bufs=3))
        obp = ctx.enter_context(tc.tile_pool(name="obp", bufs=3))
        mm = ctx.enter_context(tc.tile_pool(name="mm", bufs=2, space="PSUM"))
        spl = ctx.enter_context(tc.tile_pool(name="spl", bufs=2, space="PSUM"))
        otp = ctx.enter_context(tc.tile_pool(name="otp", bufs=1, space="PSUM"))

        # ---- persistent SBUF tensors ----
        xw = [const.tile([P, EC, 512], BF16, tag=f"xw{w}", name=f"xw{w}")
              for w in range(NW)]
        wq_s = const.tile([P, EC, QL], BF16, tag="wq")
        wkv_s = const.tile([P, EC, 2 * KVL], BF16, tag="wkv")
        wo_s = const.tile([P, QL // P, E], BF16, tag="wo")
        tri_s = const.tile([P, P], BF16, tag="tri")
        ones_s = const.tile([P, P], BF16, tag="ones")
        kt_s = const.tile([P, T], BF16, tag="kt")              # K^T (kv chans on parts)
        vag_s = const.tile([P, T // P, 2, 66], BF16, tag="vag")  # V_aug per (tchunk, half)
        warm = const.tile([1, 16], F32, tag="warm")
        warm2 = const.tile([1, 16], F32, tag="warm2")

        xT_r = xT.rearrange("(o p) t -> p o t", p=P)
        for w in range(NW):
            nc.sync.dma_start(out=xw[w], in_=xT_r[:, :, w * 512:(w + 1) * 512])
        nc.sync.dma_start(out=wq_s, in_=wqT.rearrange("(o p) q -> p o q", p=P))
        nc.sync.dma_start(out=wkv_s, in_=wkvT.rearrange("(o p) c -> p o c", p=P))
        nc.sync.dma_start(out=wo_s, in_=woT.rearrange("(o p) e -> p o e", p=P))
        nc.sync.dma_start(out=tri_s, in_=tri[:])
        # ones column of V_aug (col 64) for both halves; col 65 is pad
        nc.vector.memset(vag_s[:, :, :, 64:66], 1.0)
        nc.vector.memset(ones_s, 1.0)
        # pre-warm the exp table set so the ~2.7us ACT_TABLE_LOAD overlaps DMAs
        nc.vector.memset(warm, 0.0)
        nc.scalar.activation(out=warm2, in_=warm,
                             func=mybir.ActivationFunctionType.Exp, scale=1.0)

        qt_r = [qtp.tile([P, G, 512], BF16, tag="qt", name=f"qtr{i}")
                for i in range(4)]
        at_r = [atp.tile([P, G, 512], BF16, tag="at", name=f"atr{i}")
                for i in range(4)]

        # ---------- projection generators (background PE work) ----------
        def gen_qkv(w):
            # Q projection for window w: psum [128 qch, 512 t] per g
            for g in range(G):
                ps = mm.tile([P, 512], F32, tag="ps")
                for e in range(EC):
                    nc.tensor.matmul(
                        ps,
                        lhsT=wq_s[:, e, g * P:(g + 1) * P],
                        rhs=xw[w][:, e, :],
                        start=(e == 0),
                        stop=(e == EC - 1),
                    )
                    if e % 4 == 3:
                        yield
                nc.vector.tensor_copy(out=qt_r[w][:, g, :], in_=ps)
            # K projection for window w
            ps = mm.tile([P, 512], F32, tag="ps")
            for e in range(EC):
                nc.tensor.matmul(
                    ps, lhsT=wkv_s[:, e, 0:KVL], rhs=xw[w][:, e, :],
                    start=(e == 0), stop=(e == EC - 1),
                )
                if e % 4 == 3:
                    yield
            nc.vector.tensor_copy(out=kt_s[:, w * 512:(w + 1) * 512], in_=ps)
            # V projection for window w (natural layout, t on partitions)
            for t in range(4):
                ps = mm.tile([P, 512], F32, tag="ps")
                psv = ps[:, 0:KVL]
                for e in range(EC):
                    nc.tensor.matmul(
                        psv,
                        lhsT=xw[w][:, e, t * P:(t + 1) * P],
                        rhs=wkv_s[:, e, KVL:2 * KVL],
                        start=(e == 0),
                        stop=(e == EC - 1),
                    )
                    if e % 4 == 3:
                        yield
                tb = 4 * w + t
                nc.vector.tensor_copy(out=vag_s[:, tb, 0, 0:HD], in_=psv[:, 0:HD])
                nc.vector.tensor_copy(out=vag_s[:, tb, 1, 0:HD], in_=psv[:, HD:2 * HD])

        def gen_oproj(w):
            # O projection for window w: needs at_r[w % 3] complete
            src = at_r[w]
            for t in range(4):
                ob = obp.tile([P, E], BF16, tag="ob")
                for eo in range(E // 512):
                    ps = mm.tile([P, 512], F32, tag="ps")
                    for cc in range(QL // P):
                        nc.tensor.matmul(
                            ps,
                            lhsT=src[:, cc, t * P:(t + 1) * P],
                            rhs=wo_s[:, cc, eo * 512:(eo + 1) * 512],
                            start=(cc == 0),
                            stop=(cc == QL // P - 1),
                        )
                    nc.vector.tensor_copy(
                        out=ob[:, eo * 512:(eo + 1) * 512], in_=ps)
                    yield
                nc.sync.dma_start(out=out_r[:, 4 * w + t, :], in_=ob)

        # background scheduling: bga = qkv projections (hard deadline: window
        # w's projections must be fully emitted before attention(w)); bgo = O
        # projections, appended only once their window's at-tile is complete.
        # O yields are rationed so window 3 (40% of attention units) still
        # has PE filler work.
        bga = [(1, gen_qkv(1)), (2, gen_qkv(2)), (3, gen_qkv(3))]
        bgo = []
        o_budget = {0: 0, 1: 0, 2: 20, 3: 1 << 20}
        state = {"qc": 0}

        def pump(n):
            done = 0
            while bga and done < n:
                try:
                    next(bga[0][1])
                    done += 1
                except StopIteration:
                    bga.pop(0)
            qc = state["qc"]
            while bgo and done < n and o_budget.get(qc, 0) > 0:
                try:
                    next(bgo[0])
                    done += 1
                    o_budget[qc] -= 1
                except StopIteration:
                    bgo.pop(0)

        def drain_qkv(w):
            while bga and bga[0][0] <= w:
                try:
                    next(bga[0][1])
                except StopIteration:
                    bga.pop(0)

        # ---------- prologue: QKV for window 0 (dense, warms HAM) ----------
        for _ in gen_qkv(0):
            pass

        # ---------- main qc loop ----------
        pending_norm = [None]
        for qc in range(NW):
            state["qc"] = qc
            if qc > 0:
                drain_qkv(qc)
            qt_c = qt_r[qc]
            at_c = at_r[qc]
            den8 = d8p.tile([P, 2, 512], F32, tag="d8")
            nc.vector.memset(den8, 1.0)
            at_us = []
            kmax = 4 * qc + 3
            for g in range(G):
                ot0 = otp.tile([65, 512], F32, tag="ot0")
                ot1 = otp.tile([65, 512], F32, tag="ot1")
                for kb in range(kmax + 1):
                    j = kb - 4 * qc
                    c0 = max(j, 0) * P
                    s01 = spl.tile([P, 2, 512], F32, tag="s01")
                    for h in range(2):
                        pb = h * HD
                        nc.tensor.matmul(
                            s01[:, h, c0:512],
                            lhsT=kt_s[pb:pb + HD, kb * P:(kb + 1) * P],
                            rhs=qt_c[pb:pb + HD, g, c0:512],
                            start=True, stop=True,
                            tile_position=(pb, 0),
                        )
                    ptt = ptp.tile([P, 2, 512], BF16, tag="ptt")
                    nc.scalar.activation(
                        out=ptt[:, :, c0:512],
                        in_=s01[:, :, c0:512],
                        func=mybir.ActivationFunctionType.Exp,
                        scale=SCALE,
                    )
                    if j >= 0:
                        for h in range(2):
                            nc.vector.tensor_mul(
                                out=ptt[:, h, c0:c0 + P],
                                in0=ptt[:, h, c0:c0 + P],
                                in1=tri_s,
                            )
                    nc.tensor.matmul(
                        ot0[:, c0:512],
                        lhsT=vag_s[:, kb, 0, 0:65],
                        rhs=ptt[:, 0, c0:512],
                        start=(kb == 0), stop=(kb == kmax),
                        skip_group_check=True,
                    )
                    nc.tensor.matmul(
                        ot1[:, c0:512],
                        lhsT=vag_s[:, kb, 1, 0:65],
                        rhs=ptt[:, 1, c0:512],
                        start=(kb == 0), stop=(kb == kmax),
                        skip_group_check=True,
                    )
                    pump(1)
                # stash unnormalized outputs + denominators
                # (denominators live on psum partition 64; DVE keeps the
                # partition, then small DMAs redistribute to 32g so one
                # reciprocal covers all 8 heads of the window)
                den_sb = dsb.tile([65, 2, 512], F32, tag="dsb")
                nc.vector.tensor_copy(out=den_sb[64:65, 0, :], in_=ot0[64:65, :])
                nc.vector.tensor_copy(out=den_sb[64:65, 1, :], in_=ot1[64:65, :])
                nc.sync.dma_start(out=den8[32 * g:32 * g + 1, 0, :],
                                  in_=den_sb[64:65, 0, :])
                nc.sync.dma_start(out=den8[32 * g:32 * g + 1, 1, :],
                                  in_=den_sb[64:65, 1, :])
                au = atu.tile([P, 512], BF16, tag="au")
                st = stg.tile([HD, 512], BF16, tag="st")
                nc.vector.tensor_copy(out=au[0:HD, :], in_=ot0[0:HD, :])
                nc.vector.tensor_copy(out=st, in_=ot1[0:HD, :])
                nc.sync.dma_start(out=au[HD:P, :], in_=st)
                at_us.append(au)
                pump(2)
                if g == 0 and pending_norm[0] is not None:
                    pending_norm[0]()
                    pending_norm[0] = None
            # normalization for the whole window: one reciprocal over 8 denoms,
            # then rank-1 matmul broadcasts (row 32g of an all-ones lhsT), and
            # a DVE multiply with the psum broadcast as the one PSUM operand.
            # Emission is deferred into the next window's attention stream so
            # the PE never drains at a window boundary.
            rec = rcp.tile([P, 2, 512], F32, tag="rec", name="rec")
            recb = rcp.tile([P, 2, 512], BF16, tag="recb", name="recb")
            nc.vector.reciprocal(out=rec, in_=den8)
            nc.vector.tensor_copy(out=recb, in_=rec)

            def make_norm(qc=qc, recb=recb, at_us=at_us, at_c=at_c):
                def emit():
                    for g in range(G):
                        bc = mm.tile([P, 512], F32, tag="ps", name="bc")
                        nc.tensor.matmul(
                            bc[0:HD, :],
                            lhsT=ones_s[32 * g:32 * g + 1, 0:HD],
                            rhs=recb[32 * g:32 * g + 1, 0, :],
                            start=True, stop=True,
                            tile_position=(32 * g, 0),
                        )
                        nc.tensor.matmul(
                            bc[HD:P, :],
                            lhsT=ones_s[32 * g:32 * g + 1, 0:HD],
                            rhs=recb[32 * g:32 * g + 1, 1, :],
                            start=True, stop=True,
                            tile_position=(32 * g, HD),
                        )
                        nc.vector.tensor_mul(out=at_c[:, g, :],
                                             in0=at_us[g], in1=bc)
                    if qc < NW - 1:
                        bgo.append(gen_oproj(qc))
                return emit
            if qc < NW - 1:
                pending_norm[0] = make_norm()
            else:
                make_norm()()

        # ---------- epilogue ----------
        drain_qkv(NW)
        state["qc"] = 3
        while bgo:
            try:
                next(bgo[0])
            except StopIteration:
                bgo.pop(0)
        for _ in gen_oproj(3):
            pass

    nc.finalize()
    return nc


def _get_program():
    if "nc" not in _CACHE:
        _CACHE["nc"] = _build_program()
    return _CACHE["nc"]


def _prep_inputs(x, Wq, Wkv, Wo):
    bf = ml_dtypes.bfloat16
    x = np.asarray(x, dtype=np.float32)
    Wq = np.asarray(Wq, dtype=np.float32)
    Wkv = np.asarray(Wkv, dtype=np.float32)
    Wo = np.asarray(Wo, dtype=np.float32)

    # packed local channel order: chunk g holds [head g | head g+4]
    perm = []
    for g in range(G):
        perm.extend(range(g * HD, (g + 1) * HD))
        perm.extend(range((g + 4) * HD, (g + 5) * HD))
    perm = np.asarray(perm)

    tri = np.triu(np.ones((P, P), dtype=np.float32)).astype(bf)  # [k,q]=1 iff q>=k

    xTb = [np.ascontiguousarray(x[b].T).astype(bf) for b in range(NB)]
    wq_r, wkv_r, wo_r = [], [], []
    for r in range(NR):
        wq_loc = Wq[r * QL:(r + 1) * QL][perm]            # [512, E] packed
        wq_r.append(np.ascontiguousarray(wq_loc.T).astype(bf))
        k_rows = Wkv[r * KVL:(r + 1) * KVL]               # [128, E]
        v_rows = Wkv[HKV * HD + r * KVL:HKV * HD + (r + 1) * KVL]
        wkv_r.append(np.ascontiguousarray(np.concatenate([k_rows, v_rows], 0).T).astype(bf))
        wo_loc = Wo[:, r * QL:(r + 1) * QL][:, perm]      # [E, 512] packed cols
        wo_r.append(np.ascontiguousarray(wo_loc.T).astype(bf))

    in_maps = []
    for b in range(NB):
        for r in range(NR):
            in_maps.append({
                "xT": xTb[b],
                "wqT": wq_r[r],
                "wkvT": wkv_r[r],
                "woT": wo_r[r],
                "tri": tri,
            })
    return in_maps


def _run(x, Wq, Wkv, Wo, trace=False):
    nc = _get_program()
    in_maps = _prep_inputs(x, Wq, Wkv, Wo)
    res = run_bass_kernel_spmd(nc, in_maps, core_ids=list(range(8)), trace=trace)
    outs = [np.asarray(r["out"], dtype=np.float32) for r in res.results]
    full = np.stack([
        outs[0] + outs[1] + outs[2] + outs[3],
        outs[4] + outs[5] + outs[6] + outs[7],
    ]).astype(np.float32)
    return full, res


def kernel(x, Wq, Wkv, Wo):
    full, _ = _run(x, Wq, Wkv, Wo, trace=False)
    return full



# revision 7
# speedup vs baseline: 1.1160x; 1.1160x over previous
"""GQA attention kernel for Trainium2, 8 NeuronCores.

Problem: B=2, T=2048, E=2048, 32 query heads, 8 KV heads, head_dim=64, causal.
Sharding: 2 (batch) x 4 (tensor-parallel) grid. Each TP rank owns 2 KV heads
(=> 8 query heads, 512 q-channels). Wq/Wkv column-sharded, Wo row-sharded;
per-rank bf16 partial outputs are summed on host.

Design (single-pass pipelined structure over four 512-wide query windows;
Q/K/V projections for later windows and O-projections of completed windows
are generator-woven into the attention matmul stream):
 - v2 changes vs v1 (356.8us):
   * input DMAs reordered + host pre-interleaved to partition-major so the
     first Q-proj matmul starts at ~7us (xw0 + wq[g0] = 2.5MB) instead of
     waiting for all 13.7MB (~40us). wq is loaded per-g.
   * denominators gathered to partition pairs (32g, 32g+1) so ONE DVE
     reciprocal per window runs on free-size 512 (3.3us, was 6.5) and ONE
     K=2 broadcast matmul per g replaces two rank-1 ones (4 MMs/window vs 8).
   * window-3 normalize is sliced per t-block and interleaved with its
     O-projection; O budget rationed so the PE has filler work during the
     final reciprocal instead of idling into a HAM re-throttle.
 - score matmuls for the two KV halves run CONCURRENTLY in the PE array via
   row tiling (tile_position (0,0)/(64,0); contraction = head_dim = 64).
 - exp batched per k-block across both halves (one ACTIVATE on [128, 2x512]),
   diagonal blocks sliced to live columns; causal tri-mask on DVE.
 - attention outputs accumulate into PSUM [65,512] per half with a ones
   column producing softmax denominators for free.
 - output written as bf16 partials (halves output DMA traffic).
"""

import numpy as np
import ml_dtypes

import concourse.bass as bass
import concourse.mybir as mybir
import concourse.tile as tile
from concourse import bacc
from concourse.bass_utils import run_bass_kernel_spmd

E = 2048
T = 2048
HQ = 32
HKV = 8
HD = 64
G = 4            # query heads per kv head
P = 128
QL = 512         # local q channels per rank (8 heads)
KVL = 128        # local k (or v) channels per rank (2 heads)
NB = 2           # batches
NR = 4           # tensor-parallel ranks
SCALE = 1.0 / 8.0
EC = E // P      # 16 contraction chunks
NW = 4           # number of 512-wide t windows

BF16 = mybir.dt.bfloat16
F32 = mybir.dt.float32

_CACHE = {}


def _build_program():
    from contextlib import ExitStack

    nc = bacc.Bacc(None, target_bir_lowering=False, debug=False)
    # all inputs are host pre-interleaved to partition-major layouts so every
    # DMA lands contiguously per partition.
    xT = nc.declare_dram_parameter("xT", [P, NW, EC, 512], BF16, isOutput=False)
    wqT = nc.declare_dram_parameter("wqT", [P, G, EC, P], BF16, isOutput=False)
    wkvT = nc.declare_dram_parameter("wkvT", [P, EC, 2 * KVL], BF16, isOutput=False)
    woT = nc.declare_dram_parameter("woT", [P, QL // P, E], BF16, isOutput=False)
    tri = nc.declare_dram_parameter("tri", [P, P], BF16, isOutput=False)
    e2d = nc.declare_dram_parameter("e2d", [P, P], BF16, isOutput=False)
    out = nc.declare_dram_parameter("out", [T, E], BF16, isOutput=True)
    out_r = out.rearrange("(o p) e -> p o e", p=P)

    with tile.TileContext(nc) as tc, ExitStack() as ctx:
        const = ctx.enter_context(tc.tile_pool(name="const", bufs=1))
        qtp = ctx.enter_context(tc.tile_pool(name="qtp", bufs=4))
        atp = ctx.enter_context(tc.tile_pool(name="atp", bufs=4))
        atu = ctx.enter_context(tc.tile_pool(name="atu", bufs=9))
        stg = ctx.enter_context(tc.tile_pool(name="stg", bufs=3))
        ptp = ctx.enter_context(tc.tile_pool(name="ptp", bufs=3))
        dsb = ctx.enter_context(tc.tile_pool(name="dsb", bufs=3))
        d8p = ctx.enter_context(tc.tile_pool(name="d8p", bufs=2))
        rcp = ctx.enter_context(tc.tile_pool(name="rcp", bufs=2))
        bcp = ctx.enter_context(tc.tile_pool(name="bcp", bufs=3))
        bsp = ctx.enter_context(tc.tile_pool(name="bsp", bufs=2))
        obp = ctx.enter_context(tc.tile_pool(name="obp", bufs=3))
        mm = ctx.enter_context(tc.tile_pool(name="mm", bufs=2, space="PSUM"))
        spl = ctx.enter_context(tc.tile_pool(name="spl", bufs=2, space="PSUM"))
        otp = ctx.enter_context(tc.tile_pool(name="otp", bufs=1, space="PSUM"))

        # ---- persistent SBUF tensors ----
        xw = [const.tile([P, EC, 512], BF16, tag=f"xw{w}", name=f"xw{w}")
              for w in range(NW)]
        wq_s = const.tile([P, G, EC, P], BF16, tag="wq")
        wkv_s = const.tile([P, EC, 2 * KVL], BF16, tag="wkv")
        wo_s = const.tile([P, QL // P, E], BF16, tag="wo")
        tri_s = const.tile([P, P], BF16, tag="tri")
        e2_s = const.tile([P, P], BF16, tag="e2")
        kt_s = const.tile([P, T], BF16, tag="kt")              # K^T (kv chans on parts)
        vag_s = const.tile([P, T // P, 2, 66], BF16, tag="vag")  # V_aug per (tchunk, half)
        warm = const.tile([1, 16], F32, tag="warm")
        warm2 = const.tile([1, 16], F32, tag="warm2")

        # DMA issue order = arrival order (single sync queue): fetch exactly
        # what the first Q-proj chain needs first, stream the rest behind it.
        nc.sync.dma_start(out=xw[0], in_=xT[:, 0])
        for g in range(G):
            nc.sync.dma_start(out=wq_s[:, g], in_=wqT[:, g])
        nc.sync.dma_start(out=wkv_s, in_=wkvT[:])
        nc.sync.dma_start(out=tri_s, in_=tri[:])
        for w in range(1, NW):
            nc.sync.dma_start(out=xw[w], in_=xT[:, w])
        nc.sync.dma_start(out=wo_s, in_=woT[:])
        nc.sync.dma_start(out=e2_s, in_=e2d[:])
        # ones column of V_aug (col 64) for both halves; col 65 is pad
        nc.vector.memset(vag_s[:, :, :, 64:66], 1.0)
        # pre-warm the exp table set so the ~2.7us ACT_TABLE_LOAD overlaps DMAs
        nc.vector.memset(warm, 0.0)
        nc.scalar.activation(out=warm2, in_=warm,
                             func=mybir.ActivationFunctionType.Exp, scale=1.0)

        qt_r = [qtp.tile([P, G, 512], BF16, tag="qt", name=f"qtr{i}")
                for i in range(4)]
        at_r = [atp.tile([P, G, 512], BF16, tag="at", name=f"atr{i}")
                for i in range(4)]

        # ---------- projection generators (background PE work) ----------
        def gen_qkv(w):
            # Q projection for window w: psum [128 qch, 512 t] per g
            for g in range(G):
                ps = mm.tile([P, 512], F32, tag="ps")
                for e in range(EC):
                    nc.tensor.matmul(
                        ps,
                        lhsT=wq_s[:, g, e, :],
                        rhs=xw[w][:, e, :],
                        start=(e == 0),
                        stop=(e == EC - 1),
                    )
                    if e % 4 == 3:
                        yield
                nc.vector.tensor_copy(out=qt_r[w][:, g, :], in_=ps)
            # K projection for window w
            ps = mm.tile([P, 512], F32, tag="ps")
            for e in range(EC):
                nc.tensor.matmul(
                    ps, lhsT=wkv_s[:, e, 0:KVL], rhs=xw[w][:, e, :],
                    start=(e == 0), stop=(e == EC - 1),
                )
                if e % 4 == 3:
                    yield
            nc.vector.tensor_copy(out=kt_s[:, w * 512:(w + 1) * 512], in_=ps)
            # V projection for window w (natural layout, t on partitions)
            for t in range(4):
                ps = mm.tile([P, 512], F32, tag="ps")
                psv = ps[:, 0:KVL]
                for e in range(EC):
                    nc.tensor.matmul(
                        psv,
                        lhsT=xw[w][:, e, t * P:(t + 1) * P],
                        rhs=wkv_s[:, e, KVL:2 * KVL],
                        start=(e == 0),
                        stop=(e == EC - 1),
                    )
                    if e % 4 == 3:
                        yield
                tb = 4 * w + t
                nc.vector.tensor_copy(out=vag_s[:, tb, 0, 0:HD], in_=psv[:, 0:HD])
                nc.vector.tensor_copy(out=vag_s[:, tb, 1, 0:HD], in_=psv[:, HD:2 * HD])

        def gen_oproj(w, t_list=None):
            # O projection for window w: needs at_r[w] complete (or the
            # t-blocks listed in t_list normalized)
            src = at_r[w]
            for t in (range(4) if t_list is None else t_list):
                ob = obp.tile([P, E], BF16, tag="ob")
                for eo in range(E // 512):
                    ps = mm.tile([P, 512], F32, tag="ps")
                    for cc in range(QL // P):
                        nc.tensor.matmul(
                            ps,
                            lhsT=src[:, cc, t * P:(t + 1) * P],
                            rhs=wo_s[:, cc, eo * 512:(eo + 1) * 512],
                            start=(cc == 0),
                            stop=(cc == QL // P - 1),
                        )
                    nc.vector.tensor_copy(
                        out=ob[:, eo * 512:(eo + 1) * 512], in_=ps)
                    yield
                nc.sync.dma_start(out=out_r[:, 4 * w + t, :], in_=ob)

        # background scheduling: bga = qkv projections (hard deadline: window
        # w's projections must be fully emitted before attention(w)); bgo = O
        # projections, appended only once their window's at-tile is complete.
        # O yields are rationed so window 3 (40% of attention units) still
        # has PE filler work AND ~8 yields remain to cover the final
        # reciprocal/normalize latency in the epilogue.
        bga = [(1, gen_qkv(1)), (2, gen_qkv(2)), (3, gen_qkv(3))]
        bgo = []
        o_budget = {0: 0, 1: 0, 2: 20, 3: 20}
        state = {"qc": 0}

        def pump(n):
            done = 0
            while bga and done < n:
                try:
                    next(bga[0][1])
                    done += 1
                except StopIteration:
                    bga.pop(0)
            qc = state["qc"]
            while bgo and done < n and o_budget.get(qc, 0) > 0:
                try:
                    next(bgo[0])
                    done += 1
                    o_budget[qc] -= 1
                except StopIteration:
                    bgo.pop(0)

        def drain_qkv(w):
            while bga and bga[0][0] <= w:
                try:
                    next(bga[0][1])
                except StopIteration:
                    bga.pop(0)

        # ---------- prologue: QKV for window 0 (dense, warms HAM) ----------
        for _ in gen_qkv(0):
            pass

        # ---------- main qc loop ----------
        pending_norm = [None]
        for qc in range(NW):
            state["qc"] = qc
            if qc > 0:
                drain_qkv(qc)
            qt_c = qt_r[qc]
            at_c = at_r[qc]
            # denominators for head-group g land on partitions (32g, 32g+1):
            # one reciprocal of free-size 512 covers the whole window, and a
            # single K=2 matmul per g broadcasts both halves.
            den8 = d8p.tile([P, 512], F32, tag="d8")
            nc.vector.memset(den8, 1.0)
            at_us = []
            kmax = 4 * qc + 3
            for g in range(G):
                ot0 = otp.tile([65, 512], F32, tag="ot0")
                ot1 = otp.tile([65, 512], F32, tag="ot1")
                for kb in range(kmax + 1):
                    j = kb - 4 * qc
                    c0 = max(j, 0) * P
                    s01 = spl.tile([P, 2, 512], F32, tag="s01")
                    for h in range(2):
                        pb = h * HD
                        nc.tensor.matmul(
                            s01[:, h, c0:512],
                            lhsT=kt_s[pb:pb + HD, kb * P:(kb + 1) * P],
                            rhs=qt_c[pb:pb + HD, g, c0:512],
                            start=True, stop=True,
                            tile_position=(pb, 0),
                        )
                    ptt = ptp.tile([P, 2, 512], BF16, tag="ptt")
                    nc.scalar.activation(
                        out=ptt[:, :, c0:512],
                        in_=s01[:, :, c0:512],
                        func=mybir.ActivationFunctionType.Exp,
                        scale=SCALE,
                    )
                    if j >= 0:
                        for h in range(2):
                            nc.vector.tensor_mul(
                                out=ptt[:, h, c0:c0 + P],
                                in0=ptt[:, h, c0:c0 + P],
                                in1=tri_s,
                            )
                    nc.tensor.matmul(
                        ot0[:, c0:512],
                        lhsT=vag_s[:, kb, 0, 0:65],
                        rhs=ptt[:, 0, c0:512],
                        start=(kb == 0), stop=(kb == kmax),
                        skip_group_check=True,
                    )
                    nc.tensor.matmul(
                        ot1[:, c0:512],
                        lhsT=vag_s[:, kb, 1, 0:65],
                        rhs=ptt[:, 1, c0:512],
                        start=(kb == 0), stop=(kb == kmax),
                        skip_group_check=True,
                    )
                    pump(1)
                # stash unnormalized outputs + denominators
                # (denominators live on psum partition 64; DVE keeps the
                # partition, then small DMAs redistribute to (32g, 32g+1) so
                # one reciprocal covers all 8 heads of the window)
                den_sb = dsb.tile([65, 2, 512], F32, tag="dsb")
                nc.vector.tensor_copy(out=den_sb[64:65, 0, :], in_=ot0[64:65, :])
                nc.vector.tensor_copy(out=den_sb[64:65, 1, :], in_=ot1[64:65, :])
                nc.sync.dma_start(out=den8[32 * g:32 * g + 1, :],
                                  in_=den_sb[64:65, 0, :])
                nc.sync.dma_start(out=den8[32 * g + 1:32 * g + 2, :],
                                  in_=den_sb[64:65, 1, :])
                au = atu.tile([P, 512], BF16, tag="au")
                st = stg.tile([HD, 512], BF16, tag="st")
                nc.vector.tensor_copy(out=au[0:HD, :], in_=ot0[0:HD, :])
                nc.vector.tensor_copy(out=st, in_=ot1[0:HD, :])
                nc.sync.dma_start(out=au[HD:P, :], in_=st)
                at_us.append(au)
                pump(2)
                if g == 0 and pending_norm[0] is not None:
                    pending_norm[0]()
                    pending_norm[0] = None
            # normalization for the whole window: one reciprocal over 8 denoms
            # ([128, 512], only partition pairs 32g/32g+1 meaningful), then one
            # K=2 matmul broadcast per g, and a DVE multiply with the psum
            # broadcast as the one allowed PSUM operand. Emission is deferred
            # into the next window's attention stream so the PE never drains
            # at a window boundary.
            rec = rcp.tile([P, 512], F32, tag="rec", name="rec")
            recb = rcp.tile([P, 512], BF16, tag="recb", name="recb")
            nc.vector.reciprocal(out=rec, in_=den8)
            nc.vector.tensor_copy(out=recb, in_=rec)

            def make_norm(qc=qc, recb=recb, at_us=at_us, at_c=at_c):
                def emit():
                    for g in range(G):
                        bc = mm.tile([P, 512], F32, tag="ps", name="bc")
                        nc.tensor.matmul(
                            bc,
                            lhsT=e2_s[32 * g:32 * g + 2, 0:P],
                            rhs=recb[32 * g:32 * g + 2, :],
                            start=True, stop=True,
                            tile_position=(32 * g, 0),
                        )
                        nc.vector.tensor_mul(out=at_c[:, g, :],
                                             in0=at_us[g], in1=bc)
                    bgo.append(gen_oproj(qc))
                return emit
            if qc < NW - 1:
                pending_norm[0] = make_norm()
            else:
                # drain leftover rationed O work first: it has no dependency
                # on the reciprocal, so the PE stays busy while DVE computes
                # it (psum pool rotation orders MMs by emission, so this must
                # be emitted before the normalize chain).
                while bgo:
                    try:
                        next(bgo[0])
                    except StopIteration:
                        bgo.pop(0)
                # window 3: broadcast to SBUF once, then normalize t-block by
                # t-block, starting that t-block's O projection immediately.
                bcs = bsp.tile([P, G, 512], BF16, tag="bcs")
                for g in range(G):
                    bc = mm.tile([P, 512], F32, tag="ps", name="bc3")
                    nc.tensor.matmul(
                        bc,
                        lhsT=e2_s[32 * g:32 * g + 2, 0:P],
                        rhs=recb[32 * g:32 * g + 2, :],
                        start=True, stop=True,
                        tile_position=(32 * g, 0),
                    )
                    nc.vector.tensor_copy(out=bcs[:, g, :], in_=bc)
                for t in range(4):
                    sl = slice(t * P, (t + 1) * P)
                    for g in range(G):
                        nc.vector.tensor_mul(out=at_c[:, g, sl],
                                             in0=at_us[g][:, sl],
                                             in1=bcs[:, g, sl])
                    # drain remaining rationed O work plus this t-block
                    for _ in gen_oproj(3, t_list=[t]):
                        pump(1)

        # ---------- epilogue ----------
        drain_qkv(NW)
        state["qc"] = 3
        while bgo:
            try:
                next(bgo[0])
            except StopIteration:
                bgo.pop(0)

    nc.finalize()
    return nc


def _get_program():
    if "nc" not in _CACHE:
        _CACHE["nc"] = _build_program()
    return _CACHE["nc"]


def _prep_inputs(x, Wq, Wkv, Wo):
    bf = ml_dtypes.bfloat16
    x = np.asarray(x, dtype=np.float32)
    Wq = np.asarray(Wq, dtype=np.float32)
    Wkv = np.asarray(Wkv, dtype=np.float32)
    Wo = np.asarray(Wo, dtype=np.float32)

    # packed local channel order: chunk g holds [head g | head g+4]
    perm = []
    for g in range(G):
        perm.extend(range(g * HD, (g + 1) * HD))
        perm.extend(range((g + 4) * HD, (g + 5) * HD))
    perm = np.asarray(perm)

    tri = np.triu(np.ones((P, P), dtype=np.float32)).astype(bf)  # [k,q]=1 iff q>=k
    # e2 rows (32g, 32g+1): selector for the merged K=2 denominator broadcast
    # (row 32g covers out partitions 0:64, row 32g+1 covers 64:128)
    e2 = np.zeros((P, P), dtype=np.float32)
    for g in range(G):
        e2[32 * g, 0:HD] = 1.0
        e2[32 * g + 1, HD:P] = 1.0
    e2 = e2.astype(bf)

    # xT pre-interleaved: [P, NW, EC, 512]; element (p, w, o, t) = x[t + 512w, o*128+p]
    xTb = []
    for b in range(NB):
        xt = np.ascontiguousarray(x[b].T)                  # [E, T]
        xt = xt.reshape(EC, P, NW, 512).transpose(1, 2, 0, 3)
        xTb.append(np.ascontiguousarray(xt).astype(bf))
    wq_r, wkv_r, wo_r = [], [], []
    for r in range(NR):
        wq_loc = Wq[r * QL:(r + 1) * QL][perm]             # [512, E] packed
        wqt = wq_loc.T.reshape(EC, P, G, P).transpose(1, 2, 0, 3)  # [P, G, EC, P]
        wq_r.append(np.ascontiguousarray(wqt).astype(bf))
        k_rows = Wkv[r * KVL:(r + 1) * KVL]                # [128, E]
        v_rows = Wkv[HKV * HD + r * KVL:HKV * HD + (r + 1) * KVL]
        wkvt = np.concatenate([k_rows, v_rows], 0).T       # [E, 256]
        wkvt = wkvt.reshape(EC, P, 2 * KVL).transpose(1, 0, 2)     # [P, EC, 256]
        wkv_r.append(np.ascontiguousarray(wkvt).astype(bf))
        wo_loc = Wo[:, r * QL:(r + 1) * QL][:, perm]       # [E, 512] packed cols
        wot = wo_loc.T.reshape(QL // P, P, E).transpose(1, 0, 2)   # [P, 4, E]
        wo_r.append(np.ascontiguousarray(wot).astype(bf))

    in_maps = []
    for b in range(NB):
        for r in range(NR):
            in_maps.append({
                "xT": xTb[b],
                "wqT": wq_r[r],
                "wkvT": wkv_r[r],
                "woT": wo_r[r],
                "tri": tri,
                "e2d": e2,
            })
    return in_maps


def _run(x, Wq, Wkv, Wo, trace=False):
    nc = _get_program()
    in_maps = _prep_inputs(x, Wq, Wkv, Wo)
    res = run_bass_kernel_spmd(nc, in_maps, core_ids=list(range(8)), trace=trace)
    outs = [np.asarray(r["out"], dtype=np.float32) for r in res.results]
    full = np.stack([
        outs[0] + outs[1] + outs[2] + outs[3],
        outs[4] + outs[5] + outs[6] + outs[7],
    ]).astype(np.float32)
    return full, res


def kernel(x, Wq, Wkv, Wo):
    full, _ = _run(x, Wq, Wkv, Wo, trace=False)
    return full


# revision 16
# speedup vs baseline: 1.1419x; 1.0231x over previous
"""GQA attention kernel for Trainium2, 8 NeuronCores.

Problem: B=2, T=2048, E=2048, 32 query heads, 8 KV heads, head_dim=64, causal.
Sharding: 2 (batch) x 4 (tensor-parallel) grid. Each TP rank owns 2 KV heads
(=> 8 query heads, 512 q-channels). Wq/Wkv column-sharded, Wo row-sharded;
per-rank bf16 partial outputs are summed on host.

Design (single-pass pipelined structure over four 512-wide query windows;
Q/K/V projections for later windows and O-projections of completed windows
are generator-woven into the attention matmul stream):
 - v2 changes vs v1 (356.8us):
   * input DMAs reordered + host pre-interleaved to partition-major so the
     first Q-proj matmul starts at ~7us (xw0 + wq[g0] = 2.5MB) instead of
     waiting for all 13.7MB (~40us). wq is loaded per-g.
   * denominators gathered to partition pairs (32g, 32g+1) so ONE DVE
     reciprocal per window runs on free-size 512 (3.3us, was 6.5) and ONE
     K=2 broadcast matmul per g replaces two rank-1 ones (4 MMs/window vs 8).
   * window-3 normalize is sliced per t-block and interleaved with its
     O-projection; O budget rationed so the PE has filler work during the
     final reciprocal instead of idling into a HAM re-throttle.
 - score matmuls for the two KV halves run CONCURRENTLY in the PE array via
   row tiling (tile_position (0,0)/(64,0); contraction = head_dim = 64).
 - exp batched per k-block across both halves (one ACTIVATE on [128, 2x512]),
   diagonal blocks sliced to live columns; causal tri-mask on DVE.
 - attention outputs accumulate into PSUM [65,512] per half with a ones
   column producing softmax denominators for free.
 - output written as bf16 partials (halves output DMA traffic).
"""

import numpy as np
import ml_dtypes

import concourse.bass as bass
import concourse.mybir as mybir
import concourse.tile as tile
from concourse import bacc
from concourse.bass_utils import run_bass_kernel_spmd
from concourse.dve_ops import RECIPROCAL_APPROX_FAST, RECIP_APPROX_FAST_CONSTS

E = 2048
T = 2048
HQ = 32
HKV = 8
HD = 64
G = 4            # query heads per kv head
P = 128
QL = 512         # local q channels per rank (8 heads)
KVL = 128        # local k (or v) channels per rank (2 heads)
NB = 2           # batches
NR = 4           # tensor-parallel ranks
SCALE = 1.0 / 8.0
EC = E // P      # 16 contraction chunks
NW = 4           # number of 512-wide t windows

BF16 = mybir.dt.bfloat16
F32 = mybir.dt.float32

_CACHE = {}


def _build_program():
    from contextlib import ExitStack

    nc = bacc.Bacc(None, target_bir_lowering=False, debug=False)
    # all inputs are host pre-interleaved to partition-major layouts so every
    # DMA lands contiguously per partition.
    xT = nc.declare_dram_parameter("xT", [P, NW, EC, 512], BF16, isOutput=False)
    wqT = nc.declare_dram_parameter("wqT", [P, G, EC, P], BF16, isOutput=False)
    wkvT = nc.declare_dram_parameter("wkvT", [P, EC, 2 * KVL], BF16, isOutput=False)
    woT = nc.declare_dram_parameter("woT", [P, QL // P, E], BF16, isOutput=False)
    tri = nc.declare_dram_parameter("tri", [P, P], BF16, isOutput=False)
    e2d = nc.declare_dram_parameter("e2d", [P, P], BF16, isOutput=False)
    out = nc.declare_dram_parameter("out", [T, E], BF16, isOutput=True)
    out_r = out.rearrange("(o p) e -> p o e", p=P)

    with tile.TileContext(nc) as tc, ExitStack() as ctx:
        const = ctx.enter_context(tc.tile_pool(name="const", bufs=1))
        qtp = ctx.enter_context(tc.tile_pool(name="qtp", bufs=4))
        atp = ctx.enter_context(tc.tile_pool(name="atp", bufs=4))
        atu = ctx.enter_context(tc.tile_pool(name="atu", bufs=9))
        stg = ctx.enter_context(tc.tile_pool(name="stg", bufs=3))
        ptp = ctx.enter_context(tc.tile_pool(name="ptp", bufs=3))
        dsb = ctx.enter_context(tc.tile_pool(name="dsb", bufs=3))
        d8p = ctx.enter_context(tc.tile_pool(name="d8p", bufs=2))
        rcp = ctx.enter_context(tc.tile_pool(name="rcp", bufs=2))
        bcp = ctx.enter_context(tc.tile_pool(name="bcp", bufs=3))
        obp = ctx.enter_context(tc.tile_pool(name="obp", bufs=3))
        mm = ctx.enter_context(tc.tile_pool(name="mm", bufs=2, space="PSUM"))
        spl = ctx.enter_context(tc.tile_pool(name="spl", bufs=2, space="PSUM"))
        otp = ctx.enter_context(tc.tile_pool(name="otp", bufs=1, space="PSUM"))

        # ---- persistent SBUF tensors ----
        xw = [const.tile([P, EC, 512], BF16, tag=f"xw{w}", name=f"xw{w}")
              for w in range(NW)]
        wq_s = const.tile([P, G, EC, P], BF16, tag="wq")
        wkv_s = const.tile([P, EC, 2 * KVL], BF16, tag="wkv")
        wo_s = const.tile([P, QL // P, E], BF16, tag="wo")
        tri_s = const.tile([P, P], BF16, tag="tri")
        e2_s = const.tile([P, P], BF16, tag="e2")
        kt_s = const.tile([P, T], BF16, tag="kt")              # K^T (kv chans on parts)
        vag_s = const.tile([P, T // P, 2, 66], BF16, tag="vag")  # V_aug per (tchunk, half)
        warm = const.tile([1, 16], F32, tag="warm")
        warm2 = const.tile([1, 16], F32, tag="warm2")

        # DMA issue order = arrival order (single sync queue): fetch exactly
        # what the first Q-proj chain needs first, stream the rest behind it.
        # xw0 split in half so the first accumulation chain starts sooner.
        nc.sync.dma_start(out=xw[0][:, 0:8], in_=xT[:, 0, 0:8])
        nc.sync.dma_start(out=wq_s[:, 0], in_=wqT[:, 0])
        nc.sync.dma_start(out=xw[0][:, 8:16], in_=xT[:, 0, 8:16])
        for g in range(1, G):
            nc.sync.dma_start(out=wq_s[:, g], in_=wqT[:, g])
        nc.sync.dma_start(out=wkv_s, in_=wkvT[:])
        nc.sync.dma_start(out=tri_s, in_=tri[:])
        for w in range(1, NW):
            nc.sync.dma_start(out=xw[w], in_=xT[:, w])
        nc.sync.dma_start(out=wo_s, in_=woT[:])
        nc.sync.dma_start(out=e2_s, in_=e2d[:])
        # ones column of V_aug (col 64) for both halves; col 65 is pad
        nc.vector.memset(vag_s[:, :, :, 64:66], 1.0)
        # pre-warm the exp table set so the ~2.7us ACT_TABLE_LOAD overlaps DMAs
        nc.vector.memset(warm, 0.0)
        nc.scalar.activation(out=warm2, in_=warm,
                             func=mybir.ActivationFunctionType.Exp, scale=1.0)

        qt_r = [qtp.tile([P, G, 512], BF16, tag="qt", name=f"qtr{i}")
                for i in range(4)]
        at_r = [atp.tile([P, G, 512], BF16, tag="at", name=f"atr{i}")
                for i in range(4)]

        # ---------- projection generators (background PE work) ----------
        def gen_qkv(w):
            # Q projection for window w: psum [128 qch, 512 t] per g
            for g in range(G):
                ps = mm.tile([P, 512], F32, tag="ps")
                for e in range(EC):
                    nc.tensor.matmul(
                        ps,
                        lhsT=wq_s[:, g, e, :],
                        rhs=xw[w][:, e, :],
                        start=(e == 0),
                        stop=(e == EC - 1),
                    )
                    if e % 4 == 3:
                        yield
                nc.vector.tensor_copy(out=qt_r[w][:, g, :], in_=ps)
            # K projection for window w
            ps = mm.tile([P, 512], F32, tag="ps")
            for e in range(EC):
                nc.tensor.matmul(
                    ps, lhsT=wkv_s[:, e, 0:KVL], rhs=xw[w][:, e, :],
                    start=(e == 0), stop=(e == EC - 1),
                )
                if e % 4 == 3:
                    yield
            nc.vector.tensor_copy(out=kt_s[:, w * 512:(w + 1) * 512], in_=ps)
            # V projection for window w (natural layout, t on partitions)
            for t in range(4):
                ps = mm.tile([P, 512], F32, tag="ps")
                psv = ps[:, 0:KVL]
                for e in range(EC):
                    nc.tensor.matmul(
                        psv,
                        lhsT=xw[w][:, e, t * P:(t + 1) * P],
                        rhs=wkv_s[:, e, KVL:2 * KVL],
                        start=(e == 0),
                        stop=(e == EC - 1),
                    )
                    if e % 4 == 3:
                        yield
                tb = 4 * w + t
                nc.vector.tensor_copy(out=vag_s[:, tb, 0, 0:HD], in_=psv[:, 0:HD])
                nc.vector.tensor_copy(out=vag_s[:, tb, 1, 0:HD], in_=psv[:, HD:2 * HD])

        def gen_oproj(w):
            # O projection for window w: needs at_r[w] complete. In the tail
            # (attention done; state["tail"] checked at resume time) the
            # psum->sbuf casts go to the idle ACT engine so they don't queue
            # behind DVE work; out DMAs are per-eo so the last block's write
            # isn't gated on the full row.
            src = at_r[w]
            for t in range(4):
                ob = obp.tile([P, E], BF16, tag="ob")
                for eo in range(E // 512):
                    ps = mm.tile([P, 512], F32, tag="ps")
                    for cc in range(QL // P):
                        nc.tensor.matmul(
                            ps,
                            lhsT=src[:, cc, t * P:(t + 1) * P],
                            rhs=wo_s[:, cc, eo * 512:(eo + 1) * 512],
                            start=(cc == 0),
                            stop=(cc == QL // P - 1),
                        )
                    sl = slice(eo * 512, (eo + 1) * 512)
                    if state["tail"]:
                        nc.scalar.copy(out=ob[:, sl], in_=ps)
                    else:
                        nc.vector.tensor_copy(out=ob[:, sl], in_=ps)
                    nc.sync.dma_start(out=out_r[:, 4 * w + t, sl], in_=ob[:, sl])
                    yield

        # background scheduling: bga = qkv projections (hard deadline: window
        # w's projections must be fully emitted before attention(w)); bgo = O
        # projections, appended only once their window's at-tile is complete.
        # O yields are rationed so window 3 (40% of attention units) still
        # has PE filler work AND ~8 yields remain to cover the final
        # reciprocal/normalize latency in the epilogue.
        bga = [(1, gen_qkv(1)), (2, gen_qkv(2)), (3, gen_qkv(3))]
        bgo = []
        o_budget = {0: 0, 1: 0, 2: 20, 3: 20}
        state = {"qc": 0, "tail": False}

        def pump(n):
            done = 0
            while bga and done < n:
                try:
                    next(bga[0][1])
                    done += 1
                except StopIteration:
                    bga.pop(0)
            qc = state["qc"]
            while bgo and done < n and o_budget.get(qc, 0) > 0:
                try:
                    next(bgo[0])
                    done += 1
                    o_budget[qc] -= 1
                except StopIteration:
                    bgo.pop(0)

        def drain_qkv(w):
            while bga and bga[0][0] <= w:
                try:
                    next(bga[0][1])
                except StopIteration:
                    bga.pop(0)

        # ---------- prologue: QKV for window 0 (dense, warms HAM) ----------
        for _ in gen_qkv(0):
            pass

        # ---------- main qc loop ----------
        pending_norm = [None]
        for qc in range(NW):
            state["qc"] = qc
            if qc > 0:
                drain_qkv(qc)
            qt_c = qt_r[qc]
            at_c = at_r[qc]
            # denominators for head-group g land on partitions (32g, 32g+1):
            # one reciprocal of free-size 512 covers the whole window, and a
            # single K=2 matmul per g broadcasts both halves.
            den8 = d8p.tile([P, 512], F32, tag="d8")
            nc.vector.memset(den8, 1.0)
            at_us = []
            kmax = 4 * qc + 3
            for g in range(G):
                ot0 = otp.tile([65, 512], F32, tag="ot0")
                ot1 = otp.tile([65, 512], F32, tag="ot1")
                for kb in range(kmax + 1):
                    j = kb - 4 * qc
                    c0 = max(j, 0) * P
                    s01 = spl.tile([P, 2, 512], F32, tag="s01")
                    for h in range(2):
                        pb = h * HD
                        nc.tensor.matmul(
                            s01[:, h, c0:512],
                            lhsT=kt_s[pb:pb + HD, kb * P:(kb + 1) * P],
                            rhs=qt_c[pb:pb + HD, g, c0:512],
                            start=True, stop=True,
                            tile_position=(pb, 0),
                        )
                    ptt = ptp.tile([P, 2, 512], BF16, tag="ptt")
                    nc.scalar.activation(
                        out=ptt[:, :, c0:512],
                        in_=s01[:, :, c0:512],
                        func=mybir.ActivationFunctionType.Exp,
                        scale=SCALE,
                    )
                    if j >= 0:
                        for h in range(2):
                            nc.vector.tensor_mul(
                                out=ptt[:, h, c0:c0 + P],
                                in0=ptt[:, h, c0:c0 + P],
                                in1=tri_s,
                            )
                    nc.tensor.matmul(
                        ot0[:, c0:512],
                        lhsT=vag_s[:, kb, 0, 0:65],
                        rhs=ptt[:, 0, c0:512],
                        start=(kb == 0), stop=(kb == kmax),
                        skip_group_check=True,
                    )
                    nc.tensor.matmul(
                        ot1[:, c0:512],
                        lhsT=vag_s[:, kb, 1, 0:65],
                        rhs=ptt[:, 1, c0:512],
                        start=(kb == 0), stop=(kb == kmax),
                        skip_group_check=True,
                    )
                    pump(1)
                # stash unnormalized outputs + denominators
                # (denominators live on psum partition 64; DVE keeps the
                # partition, then small DMAs redistribute to (32g, 32g+1) so
                # one reciprocal covers all 8 heads of the window). For the
                # very last group these copies run on the idle ACT engine so
                # the reciprocal isn't queued behind them on DVE.
                last = (qc == NW - 1 and g == G - 1)
                cp = nc.scalar.copy if last else nc.vector.tensor_copy
                den_sb = dsb.tile([65, 2, 512], F32, tag="dsb")
                cp(out=den_sb[64:65, 0, :], in_=ot0[64:65, :])
                cp(out=den_sb[64:65, 1, :], in_=ot1[64:65, :])
                nc.sync.dma_start(out=den8[32 * g:32 * g + 1, :],
                                  in_=den_sb[64:65, 0, :])
                nc.sync.dma_start(out=den8[32 * g + 1:32 * g + 2, :],
                                  in_=den_sb[64:65, 1, :])
                au = atu.tile([P, 512], BF16, tag="au")
                st = stg.tile([HD, 512], BF16, tag="st")
                cp(out=au[0:HD, :], in_=ot0[0:HD, :])
                cp(out=st, in_=ot1[0:HD, :])
                nc.sync.dma_start(out=au[HD:P, :], in_=st)
                at_us.append(au)
                pump(2)
                if g == 0 and pending_norm[0] is not None:
                    pending_norm[0]()
                    pending_norm[0] = None
            # normalization for the whole window: one approximate reciprocal
            # (single-instruction, ~5x faster than the iterative divide, 51
            # ULP which is far below the bf16 rounding of its output) over 8
            # denoms ([128, 512], only partition pairs 32g/32g+1 meaningful),
            # then one K=2 matmul broadcast per g, and a DVE multiply with
            # the psum broadcast as the one allowed PSUM operand. Emission is
            # deferred into the next window's attention stream so the PE
            # never drains at a window boundary.
            recb = rcp.tile([P, 512], BF16, tag="recb", name="recb")

            def emit_recip(recb=recb, den8=den8):
                with nc.allow_low_precision("bf16 1/den; matches old recb cast"):
                    nc.vector._custom_dve(
                        RECIPROCAL_APPROX_FAST, out=recb, in0=den8,
                        s0=RECIP_APPROX_FAST_CONSTS["s0"],
                        s1=RECIP_APPROX_FAST_CONSTS["s1"],
                        imm2=RECIP_APPROX_FAST_CONSTS["imm2"],
                    )

            def make_norm(qc=qc, recb=recb, at_us=at_us, at_c=at_c):
                def emit():
                    for g in range(G):
                        bc = mm.tile([P, 512], F32, tag="ps", name="bc")
                        nc.tensor.matmul(
                            bc,
                            lhsT=e2_s[32 * g:32 * g + 2, 0:P],
                            rhs=recb[32 * g:32 * g + 2, :],
                            start=True, stop=True,
                            tile_position=(32 * g, 0),
                        )
                        nc.vector.tensor_mul(out=at_c[:, g, :],
                                             in0=at_us[g], in1=bc)
                    bgo.append(gen_oproj(qc))
                return emit
            if qc < NW - 1:
                emit_recip()
                pending_norm[0] = make_norm()
            else:
                # tail: the PE instruction queue is strict FIFO, so emission
                # order is placement. Drain a little leftover rationed O work
                # (no reciprocal dependency) to cover the den-DMA+reciprocal
                # latency, then broadcast from the now-idle score psum pool,
                # then the rest of the leftovers while DVE runs the muls,
                # then window 3's own O projection.
                emit_recip()
                state["tail"] = True

                def drain_o(n):
                    done = 0
                    while bgo and (n is None or done < n):
                        try:
                            next(bgo[0])
                            done += 1
                        except StopIteration:
                            bgo.pop(0)
                drain_o(2)
                for g in range(G):
                    bct = spl.tile([P, 2, 512], F32, tag="s01", name=f"bct{g}")
                    bc = bct[:, 0, :]
                    nc.tensor.matmul(
                        bc,
                        lhsT=e2_s[32 * g:32 * g + 2, 0:P],
                        rhs=recb[32 * g:32 * g + 2, :],
                        start=True, stop=True,
                        tile_position=(32 * g, 0),
                    )
                    nc.vector.tensor_mul(out=at_c[:, g, :],
                                         in0=at_us[g], in1=bc)
                drain_o(None)
                for _ in gen_oproj(3):
                    pass

        # ---------- epilogue (safety net; normally empty) ----------
        drain_qkv(NW)
        state["qc"] = 3
        while bgo:
            try:
                next(bgo[0])
            except StopIteration:
                bgo.pop(0)

    nc.finalize()
    return nc


def _get_program():
    if "nc" not in _CACHE:
        _CACHE["nc"] = _build_program()
    return _CACHE["nc"]


def _prep_inputs(x, Wq, Wkv, Wo):
    bf = ml_dtypes.bfloat16
    x = np.asarray(x, dtype=np.float32)
    Wq = np.asarray(Wq, dtype=np.float32)
    Wkv = np.asarray(Wkv, dtype=np.float32)
    Wo = np.asarray(Wo, dtype=np.float32)

    # packed local channel order: chunk g holds [head g | head g+4]
    perm = []
    for g in range(G):
        perm.extend(range(g * HD, (g + 1) * HD))
        perm.extend(range((g + 4) * HD, (g + 5) * HD))
    perm = np.asarray(perm)

    tri = np.triu(np.ones((P, P), dtype=np.float32)).astype(bf)  # [k,q]=1 iff q>=k
    # e2 rows (32g, 32g+1): selector for the merged K=2 denominator broadcast
    # (row 32g covers out partitions 0:64, row 32g+1 covers 64:128)
    e2 = np.zeros((P, P), dtype=np.float32)
    for g in range(G):
        e2[32 * g, 0:HD] = 1.0
        e2[32 * g + 1, HD:P] = 1.0
    e2 = e2.astype(bf)

    # xT pre-interleaved: [P, NW, EC, 512]; element (p, w, o, t) = x[t + 512w, o*128+p]
    xTb = []
    for b in range(NB):
        xt = np.ascontiguousarray(x[b].T)                  # [E, T]
        xt = xt.reshape(EC, P, NW, 512).transpose(1, 2, 0, 3)
        xTb.append(np.ascontiguousarray(xt).astype(bf))
    wq_r, wkv_r, wo_r = [], [], []
    for r in range(NR):
        wq_loc = Wq[r * QL:(r + 1) * QL][perm]             # [512, E] packed
        wqt = wq_loc.T.reshape(EC, P, G, P).transpose(1, 2, 0, 3)  # [P, G, EC, P]
        wq_r.append(np.ascontiguousarray(wqt).astype(bf))
        k_rows = Wkv[r * KVL:(r + 1) * KVL]                # [128, E]
        v_rows = Wkv[HKV * HD + r * KVL:HKV * HD + (r + 1) * KVL]
        wkvt = np.concatenate([k_rows, v_rows], 0).T       # [E, 256]
        wkvt = wkvt.reshape(EC, P, 2 * KVL).transpose(1, 0, 2)     # [P, EC, 256]
        wkv_r.append(np.ascontiguousarray(wkvt).astype(bf))
        wo_loc = Wo[:, r * QL:(r + 1) * QL][:, perm]       # [E, 512] packed cols
        wot = wo_loc.T.reshape(QL // P, P, E).transpose(1, 0, 2)   # [P, 4, E]
        wo_r.append(np.ascontiguousarray(wot).astype(bf))

    in_maps = []
    for b in range(NB):
        for r in range(NR):
            in_maps.append({
                "xT": xTb[b],
                "wqT": wq_r[r],
                "wkvT": wkv_r[r],
                "woT": wo_r[r],
                "tri": tri,
                "e2d": e2,
            })
    return in_maps


def _run(x, Wq, Wkv, Wo, trace=False):
    nc = _get_program()
    in_maps = _prep_inputs(x, Wq, Wkv, Wo)
    res = run_bass_kernel_spmd(nc, in_maps, core_ids=list(range(8)), trace=trace)
    outs = [np.asarray(r["out"], dtype=np.float32) for r in res.results]
    full = np.stack([
        outs[0] + outs[1] + outs[2] + outs[3],
        outs[4] + outs[5] + outs[6] + outs[7],
    ]).astype(np.float32)
    return full, res


def kernel(x, Wq, Wkv, Wo):
    full, _ = _run(x, Wq, Wkv, Wo, trace=False)
    return full
